# revision 14
# baseline (speedup 1.0000x reference)
"""Bass/Tile attention kernel for TRN2 — per-core program builder.

Sharding (SPMD, core c of 8): batch b = c//2, head-half hh = c%2.
Each core projects Q/K/V for its 8 heads (4 pairs, 512 dims) over ALL
2048 tokens of its batch, runs attention for those heads, then the two
cores of a batch AllGather their Z halves and each computes a 512-wide
e-column slice of the output projection.

Inputs (per core DRAM):
  xt  : [D, S]  bf16   X[b]^T (full token set)
  wq/wk/wv : [D, DH] bf16  column slice for this core's heads
  wo  : [D, DH] bf16       e-column slice computed by this core
  bq/bk/bv/bo : [DH] fp32  matching slices
Output:
  ot  : [DH, S] fp32   O^T e-column slice (host transposes/concats).

Layouts on chip (P=128 partitions):
  xt_sb[p, c, t]  = X^T[c*128+p, t]                 (bf16)
  kt[pr][p, t]    = K^T[pr*128+p, t]  (local pair pr = 2 heads)
  qt[pr][p, q]    = Q^T[pr*128+p, q]
  vt[pr][p, tk, h*65+j] = V[tk*128+p, pr*128+h*64+j] for j<64,
                          1.0 for j==64 (augmented ones col)   (bf16)
  zt[pr][p, q]    = Z^T[pr*128+p, q] (normalized)   (bf16)
  zp[pr][p, q]    = peer core's Z^T (via AllGather) (bf16)

Attention per (pair, 512-query chunk qc), PSUM chains per bank:
  for kc in range(S/128):
    scores^T[k,q] both heads -> PSUM [128, 2, 512] (2 banks) via
      row-tiled matmuls (d=64 contraction: head-lo rows 0-63,
      head-hi rows 64-127; concurrent sub-array execution).
    probs = exp(0.125*scores): one ACT instr [128, 1024] -> bf16 SBUF.
    PV per head, stationary [V_h | 1] (M=65): z PSUM [65, 512] chain
      accumulated over kc; row 64 = softmax denominator.
  reciprocal of row 64 via DRAM-bounce partition-broadcast; scale rows
  0-63 -> zt (head-hi via bf16 stage + partition-shift DMA).
Exchange: per pair, zt -> DRAM, AllGather over {2b, 2b+1}, peer half
  -> zp.
O-proj: O^T[e,q] = sum_d Wo[d,e] Z^T[d,q] + bo over 8 global d-chunks
  (4 local zt + 4 peer zp), evicted with bias add, DMA to ot.
"""

import numpy as np
import ml_dtypes
from contextlib import ExitStack

import concourse.bass as bass
import concourse.tile as tile
from concourse import bacc, mybir

F32 = mybir.dt.float32
BF16 = mybir.dt.bfloat16
P = 128


def build_attention_nc(S=2048, D=1024, DH=512, H=16):
    DK = D // H
    assert DK == 64
    NPL = DH // P             # local head pairs (4)
    KC = S // P               # k chunks (16)
    TOKC = S // 512           # 512-token col chunks for K proj (4)
    NTOK = S // P             # 128-token chunks for V proj (16)
    QCN = S // 512            # 512-query chunks (4)
    DIN = D // P              # input-dim chunks (8)

    nc = bacc.Bacc("TRN2", target_bir_lowering=False, debug=False,
                   num_devices=8)

    xt_d = nc.dram_tensor("xt", [D, S], BF16, kind="ExternalInput").ap()
    wq_d = nc.dram_tensor("wq", [D, DH], BF16, kind="ExternalInput").ap()
    wk_d = nc.dram_tensor("wk", [D, DH], BF16, kind="ExternalInput").ap()
    wv_d = nc.dram_tensor("wv", [D, DH], BF16, kind="ExternalInput").ap()
    wo_d = nc.dram_tensor("wo", [D, DH], BF16, kind="ExternalInput").ap()
    bq_d = nc.dram_tensor("bq", [DH], F32, kind="ExternalInput").ap()
    bk_d = nc.dram_tensor("bk", [DH], F32, kind="ExternalInput").ap()
    bv_d = nc.dram_tensor("bv", [DH], F32, kind="ExternalInput").ap()
    bo_d = nc.dram_tensor("bo", [DH], F32, kind="ExternalInput").ap()
    ot_d = nc.dram_tensor("ot", [DH, S], F32, kind="ExternalOutput").ap()
    # collective staging
    zout_d = nc.dram_tensor("zout", [NPL, P, S], BF16, kind="Internal").ap()
    # per-pair gather target must be one contiguous block: [pair, rank, ...]
    zall_d = nc.dram_tensor("zall", [NPL, 2, P, S], BF16,
                            kind="Internal").ap()

    xt_r = xt_d.rearrange("(c p) t -> p c t", p=P)
    wq_r = wq_d.rearrange("(c p) n -> p c n", p=P)
    wk_r = wk_d.rearrange("(c p) n -> p c n", p=P)
    wv_r = wv_d.rearrange("(c p) n -> p c n", p=P)
    wo_r = wo_d.rearrange("(c p) n -> p c n", p=P)
    bq_r = bq_d.rearrange("(c p) -> p c", p=P)
    bk_r = bk_d.rearrange("(c p) -> p c", p=P)
    bo_r = bo_d.rearrange("(c p) -> p c", p=P)
    bv_r = bv_d.rearrange("(a d) -> a d", a=1)

    EXP = mybir.ActivationFunctionType.Exp
    RG = [[0, 1], [2, 3], [4, 5], [6, 7]]

    with tile.TileContext(nc) as tc, ExitStack() as ctx:
        const = ctx.enter_context(tc.tile_pool(name="const", bufs=1))
        big = ctx.enter_context(tc.tile_pool(name="big", bufs=1))
        wpool = ctx.enter_context(tc.tile_pool(name="wpool", bufs=3))
        work = ctx.enter_context(tc.tile_pool(name="work", bufs=3))
        probs_pool = ctx.enter_context(tc.tile_pool(name="probs", bufs=4))
        # PSUM budget (8 banks): scores 2x2 + z 3 + proj 1
        spsum = ctx.enter_context(tc.tile_pool(name="spsum", bufs=2, space="PSUM"))
        zpsum = ctx.enter_context(tc.tile_pool(name="zpsum", bufs=3, space="PSUM"))
        ppsum = ctx.enter_context(tc.tile_pool(name="ppsum", bufs=1, space="PSUM"))
        dramp = ctx.enter_context(tc.tile_pool(name="dramp", bufs=2, space="DRAM"))

        # ---- constants ----
        bq_t = const.tile([P, NPL], F32)
        nc.sync.dma_start(bq_t[:], bq_r[:, :])
        bk_t = const.tile([P, NPL], F32)
        nc.sync.dma_start(bk_t[:], bk_r[:, :])
        bo_t = const.tile([P, NPL], F32)
        nc.sync.dma_start(bo_t[:], bo_r[:, :])
        bvb = const.tile([P, DH], F32)
        nc.sync.dma_start(bvb[:], bv_r[0:1, :].to_broadcast((P, DH)))

        # ---- X^T + weight loads, chunk-interleaved ----
        xt_t = big.tile([P, DIN, S], BF16, tag="xt")
        wk_t = wpool.tile([P, DIN, DH], BF16, tag="w", name="wk_t")
        wv_t = wpool.tile([P, DIN, DH], BF16, tag="w", name="wv_t")
        wq_t = wpool.tile([P, DIN, DH], BF16, tag="w", name="wq_t")
        for c in range(DIN):
            nc.sync.dma_start(wk_t[:, c, :], wk_r[:, c, :])
            nc.sync.dma_start(xt_t[:, c, :], xt_r[:, c, :])
            nc.sync.dma_start(wv_t[:, c, :], wv_r[:, c, :])
            nc.sync.dma_start(wq_t[:, c, :], wq_r[:, c, :])

        kt = [big.tile([P, S], BF16, tag=f"kt{p}", name=f"kt{p}")
              for p in range(NPL)]
        qt = [big.tile([P, S], BF16, tag=f"qt{p}", name=f"qt{p}")
              for p in range(NPL)]
        vt = [big.tile([P, NTOK, 130], BF16, tag=f"vt{p}", name=f"vt{p}")
              for p in range(NPL)]
        zt = [big.tile([P, S], BF16, tag=f"zt{p}", name=f"zt{p}")
              for p in range(NPL)]

        # ones columns of vt (col 64 of each head's 65-wide block)
        for pr in range(NPL):
            ones_ap = vt[pr].rearrange("p t (h c) -> p t h c", c=65)[:, :, :, 64:65]
            nc.vector.memset(ones_ap, 1.0)

        def k_proj(pr):
            for t in range(TOKC):
                ps = spsum.tile([P, 512], F32, tag="sc", name="psk")
                for c in range(DIN):
                    nc.tensor.matmul(
                        ps[:],
                        wk_t[:, c, pr * P:(pr + 1) * P],
                        xt_t[:, c, t * 512:(t + 1) * 512],
                        start=(c == 0), stop=(c == DIN - 1),
                    )
                nc.vector.tensor_scalar_add(
                    kt[pr][:, t * 512:(t + 1) * 512], ps[:], bk_t[:, pr:pr + 1]
                )

        def v_proj(g):
            # V projection for pair-group g (pairs 2g, 2g+1; 256 dims)
            prs = (2 * g, 2 * g + 1)
            for t in range(NTOK):
                ps = ppsum.tile([P, 256], F32, tag="proj", name="psv")
                for c in range(DIN):
                    nc.tensor.matmul(
                        ps[:],
                        xt_t[:, c, t * P:(t + 1) * P],
                        wv_t[:, c, g * 256:(g + 1) * 256],
                        start=(c == 0), stop=(c == DIN - 1),
                    )
                for u in range(2):
                    dst = vt[prs[u]].rearrange(
                        "p t (h c) -> p t h c", c=65)[:, t, :, 0:64]
                    src = ps[:, u * 128:(u + 1) * 128].rearrange(
                        "p (h c) -> p h c", c=64)
                    bsrc = bvb[:, (g * 256 + u * 128):
                               (g * 256 + (u + 1) * 128)
                               ].rearrange("p (h c) -> p h c", c=64)
                    nc.vector.tensor_add(dst, src, bsrc)

        def q_proj(pr):
            for t in range(QCN):
                ps = ppsum.tile([P, 512], F32, tag="proj", name="psq")
                for c in range(DIN):
                    nc.tensor.matmul(
                        ps[:],
                        wq_t[:, c, pr * P:(pr + 1) * P],
                        xt_t[:, c, t * 512:(t + 1) * 512],
                        start=(c == 0), stop=(c == DIN - 1),
                    )
                nc.vector.tensor_scalar_add(
                    qt[pr][:, t * 512:(t + 1) * 512], ps[:],
                    bq_t[:, pr:pr + 1]
                )

        def attention(pr):
            vpr = vt[pr].rearrange("p t (h c) -> p t h c", c=65)
            for qc in range(QCN):
                qsl = slice(qc * 512, (qc + 1) * 512)
                za = zpsum.tile([P, 512], F32, tag="z", name=f"za{pr}_{qc}")
                zb = zpsum.tile([P, 512], F32, tag="z", name=f"zb{pr}_{qc}")
                for kc in range(KC):
                    sq = spsum.tile([P, 2, 512], F32, tag="sc", name="sq")
                    nc.tensor.matmul(
                        sq[:, 0, :],
                        kt[pr][0:64, kc * P:(kc + 1) * P],
                        qt[pr][0:64, qsl],
                        start=True, stop=True,
                    )
                    nc.tensor.matmul(
                        sq[:, 1, :],
                        kt[pr][64:128, kc * P:(kc + 1) * P],
                        qt[pr][64:128, qsl],
                        start=True, stop=True,
                    )
                    pq = probs_pool.tile([P, 2, 512], BF16, tag="probs",
                                         name="pq")
                    nc.scalar.activation(pq[:], sq[:], EXP, scale=0.125)
                    nc.tensor.matmul(
                        za[0:65, :], vpr[:, kc, 0, :], pq[:, 0, :],
                        start=(kc == 0), stop=(kc == KC - 1),
                    )
                    nc.tensor.matmul(
                        zb[0:65, :], vpr[:, kc, 1, :], pq[:, 1, :],
                        start=(kc == 0), stop=(kc == KC - 1),
                    )
                # normalize + evict Z^T; 1/rowsum partition-broadcast
                # goes through a DRAM bounce (SBUF->SBUF bcast illegal)
                rcin = work.tile([P, 2, 512], F32, tag="rcin", name="rcin",
                                 bufs=2)
                rsc = dramp.tile([2, 512], F32, tag="rsc", name="rsc")
                nc.scalar.copy(rcin[64:65, 0, :], za[64:65, :])
                nc.vector.tensor_copy(rcin[64:65, 1, :], zb[64:65, :])
                nc.sync.dma_start(rsc[0:1, :], rcin[64:65, 0, :])
                nc.sync.dma_start(rsc[1:2, :], rcin[64:65, 1, :])
                rbr = work.tile([P, 2, 512], F32, tag="rbc", name="rbr")
                nc.sync.dma_start(rbr[0:64, 0, :],
                                  rsc[0:1, :].to_broadcast((64, 512)))
                nc.sync.dma_start(rbr[0:64, 1, :],
                                  rsc[1:2, :].to_broadcast((64, 512)))
                rb = work.tile([P, 2, 512], F32, tag="rbc", name="rb")
                nc.vector.reciprocal_approx_fast(rb[0:64, :, :],
                                                 rbr[0:64, :, :])
                nc.vector.tensor_mul(zt[pr][0:64, qsl], za[0:64, :],
                                     rb[0:64, 0, :])
                zs = work.tile([P, 512], BF16, tag="zstage", name="zs")
                nc.vector.tensor_mul(zs[0:64, :], zb[0:64, :],
                                     rb[0:64, 1, :])
                nc.sync.dma_start(zt[pr][64:128, qsl], zs[0:64, :])

        def exchange(pr):
            # publish this pair's Z^T; gather both halves for the batch.
            # zall rank order == replica-group order {2b, 2b+1} == head-half
            # order, so zall[r, pr] is global d-chunk r*NPL + pr for every
            # core — the program stays rank-independent.
            nc.sync.dma_start(zout_d[pr], zt[pr][:])
            nc.gpsimd.collective_compute(
                "AllGather",
                mybir.AluOpType.bypass,
                replica_groups=RG,
                ins=[zout_d[pr]],
                outs=[zall_d[pr]],
            )

        # ---- pipelined schedule ----
        k_proj(0)
        v_proj(0)
        q_proj(0)
        attention(0)
        exchange(0)
        k_proj(1)
        q_proj(1)
        attention(1)
        exchange(1)
        k_proj(2)
        k_proj(3)
        v_proj(1)
        wo_t = wpool.tile([P, DIN, DH], BF16, tag="w", name="wo_t")
        for c in range(DIN):
            nc.sync.dma_start(wo_t[:, c, :], wo_r[:, c, :])
        q_proj(2)
        attention(2)
        exchange(2)
        q_proj(3)
        attention(3)
        exchange(3)

        # ---- output projection (global d order: row0 pairs then row1) ----
        # zsrc[dc] for global d-chunk dc: rows 0..3 -> zall row 0 = cores
        # with hh=0 = global pairs 0..3; rows 4..7 -> hh=1 pairs.
        # For a core with hh=0: local zt are global 0..3 and zp = row 1
        # (global 4..7). For hh=1 the local zt ARE row 1; zp = row 1 as
        # well (self!). To keep the program rank-independent we use the
        # gathered zall rows for BOTH halves: row 0 via zp0 tiles, row 1
        # via zp tiles... simplest rank-independent source: always read
        # from zall (DRAM) into zg tiles for all 8 global chunks.
        zg = [big.tile([P, S], BF16, tag=f"zg{i}", name=f"zg{i}")
              for i in range(2 * NPL)]
        for r in range(2):
            for pr in range(NPL):
                nc.sync.dma_start(zg[r * NPL + pr][:], zall_d[pr, r])
        for qc in range(QCN):
            oqsl = slice(qc * 512, (qc + 1) * 512)
            for ec in range(NPL):
                ps = spsum.tile([P, 512], F32, tag="sc", name="pso")
                for dc in range(DIN):
                    nc.tensor.matmul(
                        ps[:],
                        wo_t[:, dc, ec * P:(ec + 1) * P],
                        zg[dc][:, oqsl],
                        start=(dc == 0), stop=(dc == DIN - 1),
                    )
                st = work.tile([P, 512], F32, tag="stage", name="st", bufs=2)
                nc.vector.tensor_scalar_add(st[:], ps[:], bo_t[:, ec:ec + 1])
                nc.sync.dma_start(ot_d[ec * P:(ec + 1) * P, oqsl], st[:])

    nc.compile()
    return nc



# ---------------- host-side entry point ----------------

BF = ml_dtypes.bfloat16
_B, _S, _D, _H = 4, 2048, 1024, 16
_DH = _D // 2
_NC_CACHE = None


def _get_nc():
    global _NC_CACHE
    if _NC_CACHE is None:
        _NC_CACHE = build_attention_nc(S=_S, D=_D, DH=_DH, H=_H)
    return _NC_CACHE


def kernel(X, Wq, bq, Wk, bk, Wv, bv, Wo, bo):
    """Full-input multi-head attention on 8 TRN2 NeuronCores.

    Sharding: core c handles batch c//2 and head-half c%2 (8 heads).
    The two cores of a batch AllGather Z and each computes half the
    output-projection columns. Inputs cast to bf16 on host; output is
    fp32 [B, S, D].
    """
    from concourse.bass_utils import run_bass_kernel_spmd

    X = np.asarray(X, dtype=np.float32)
    bq = np.asarray(bq, dtype=np.float32)
    bk = np.asarray(bk, dtype=np.float32)
    bv = np.asarray(bv, dtype=np.float32)
    bo = np.asarray(bo, dtype=np.float32)
    wqb = np.asarray(Wq, dtype=np.float32).astype(BF)
    wkb = np.asarray(Wk, dtype=np.float32).astype(BF)
    wvb = np.asarray(Wv, dtype=np.float32).astype(BF)
    wob = np.asarray(Wo, dtype=np.float32).astype(BF)

    in_maps = []
    for c in range(8):
        b, hh = c // 2, c % 2
        hsl = slice(hh * _DH, (hh + 1) * _DH)
        xtp = np.ascontiguousarray(X[b].T.astype(BF))
        in_maps.append({
            "xt": xtp,
            "wq": np.ascontiguousarray(wqb[:, hsl]),
            "wk": np.ascontiguousarray(wkb[:, hsl]),
            "wv": np.ascontiguousarray(wvb[:, hsl]),
            "wo": np.ascontiguousarray(wob[:, hsl]),
            "bq": np.ascontiguousarray(bq[hsl]),
            "bk": np.ascontiguousarray(bk[hsl]),
            "bv": np.ascontiguousarray(bv[hsl]),
            "bo": np.ascontiguousarray(bo[hsl]),
        })

    nc = _get_nc()
    res = run_bass_kernel_spmd(nc, in_maps, list(range(8)))

    out = np.empty((_B, _S, _D), np.float32)
    for c in range(8):
        b, hh = c // 2, c % 2
        out[b, :, hh * _DH:(hh + 1) * _DH] = res.results[c]["ot"].T
    return out


# revision 22
# speedup vs baseline: 1.0243x; 1.0243x over previous
"""Bass/Tile attention kernel for TRN2 — per-core program builder.

Sharding (SPMD, core c of 8): batch b = c//2, head-half hh = c%2.
Each core projects Q/K/V for its 8 heads (4 pairs, 512 dims) over ALL
2048 tokens of its batch, runs attention for those heads, then the two
cores of a batch AllGather their Z halves and each computes a 512-wide
e-column slice of the output projection.

Inputs (per core DRAM):
  xt  : [D, S]  bf16   X[b]^T (full token set)
  wq/wk/wv : [D, DH] bf16  column slice for this core's heads
  wo  : [DH, D] bf16       ROW slice of Wo for this core's head dims
  bq/bk/bv : [DH] fp32     matching slices
  bo  : [D] fp32           full bo on hh=0 cores, zeros on hh=1
Output:
  ot  : [D, S] fp32    partial O^T = Wo_rows^T @ Z^T (+bo); the host
                       sums the two head-halves' partials per batch
                       while unsharding (no cross-core collective).

Layouts on chip (P=128 partitions):
  xt_sb[p, c, t]  = X^T[c*128+p, t]                 (bf16)
  kt[pr][p, t]    = K^T[pr*128+p, t]  (local pair pr = 2 heads)
  qt[pr][p, q]    = Q^T[pr*128+p, q]
  vt[pr][p, tk, h*65+j] = V[tk*128+p, pr*128+h*64+j] for j<64,
                          1.0 for j==64 (augmented ones col)   (bf16)
  zt[pr][p, q]    = Z^T[pr*128+p, q] (normalized)   (bf16)
  zp[pr][p, q]    = peer core's Z^T (via AllGather) (bf16)

Attention per (pair, 512-query chunk qc), PSUM chains per bank:
  for kc in range(S/128):
    scores^T[k,q] both heads -> PSUM [128, 2, 512] (2 banks) via
      row-tiled matmuls (d=64 contraction: head-lo rows 0-63,
      head-hi rows 64-127; concurrent sub-array execution).
    probs = exp(0.125*scores): one ACT instr [128, 1024] -> bf16 SBUF.
    PV per head, stationary [V_h | 1] (M=65): z PSUM [65, 512] chain
      accumulated over kc; row 64 = softmax denominator.
  reciprocal of row 64 via DRAM-bounce partition-broadcast; scale rows
  0-63 -> zt (head-hi via bf16 stage + partition-shift DMA).
O-proj partial: ot[e,q] = sum_{d in local 512} Wo[d,e] Z^T[d,q] + bo,
  accumulated over the 4 local pair-chunks, evicted with bias add
  (bo is zeros on hh=1 cores), DMA to ot.
"""

import numpy as np
import ml_dtypes
from contextlib import ExitStack

import concourse.bass as bass
import concourse.tile as tile
from concourse import bacc, mybir

F32 = mybir.dt.float32
BF16 = mybir.dt.bfloat16
P = 128


def build_attention_nc(S=2048, D=1024, DH=512, H=16):
    DK = D // H
    assert DK == 64
    NPL = DH // P             # local head pairs (4)
    KC = S // P               # k chunks (16)
    TOKC = S // 512           # 512-token col chunks for K proj (4)
    NTOK = S // P             # 128-token chunks for V proj (16)
    QCN = S // 512            # 512-query chunks (4)
    DIN = D // P              # input-dim chunks (8)

    nc = bacc.Bacc("TRN2", target_bir_lowering=False, debug=False,
                   num_devices=8)

    xt_d = nc.dram_tensor("xt", [D, S], BF16, kind="ExternalInput").ap()
    wq_d = nc.dram_tensor("wq", [D, DH], BF16, kind="ExternalInput").ap()
    wk_d = nc.dram_tensor("wk", [D, DH], BF16, kind="ExternalInput").ap()
    wv_d = nc.dram_tensor("wv", [D, DH], BF16, kind="ExternalInput").ap()
    wo_d = nc.dram_tensor("wo", [DH, D], BF16, kind="ExternalInput").ap()
    bq_d = nc.dram_tensor("bq", [DH], F32, kind="ExternalInput").ap()
    bk_d = nc.dram_tensor("bk", [DH], F32, kind="ExternalInput").ap()
    bv_d = nc.dram_tensor("bv", [DH], F32, kind="ExternalInput").ap()
    bo_d = nc.dram_tensor("bo", [D], F32, kind="ExternalInput").ap()
    ot_d = nc.dram_tensor("ot", [D, S], F32, kind="ExternalOutput").ap()

    xt_r = xt_d.rearrange("(c p) t -> p c t", p=P)
    wq_r = wq_d.rearrange("(c p) n -> p c n", p=P)
    wk_r = wk_d.rearrange("(c p) n -> p c n", p=P)
    wv_r = wv_d.rearrange("(c p) n -> p c n", p=P)
    wo_r = wo_d.rearrange("(c p) n -> p c n", p=P)  # c = NPL row-chunks
    bq_r = bq_d.rearrange("(c p) -> p c", p=P)
    bk_r = bk_d.rearrange("(c p) -> p c", p=P)
    bo_r = bo_d.rearrange("(c p) -> p c", p=P)      # c = DIN e-chunks
    bv_r = bv_d.rearrange("(a d) -> a d", a=1)

    EXP = mybir.ActivationFunctionType.Exp

    with tile.TileContext(nc) as tc, ExitStack() as ctx:
        const = ctx.enter_context(tc.tile_pool(name="const", bufs=1))
        big = ctx.enter_context(tc.tile_pool(name="big", bufs=1))
        wpool = ctx.enter_context(tc.tile_pool(name="wpool", bufs=3))
        work = ctx.enter_context(tc.tile_pool(name="work", bufs=3))
        probs_pool = ctx.enter_context(tc.tile_pool(name="probs", bufs=4))
        # PSUM budget (8 banks): scores 2x2 + z 3 + proj 1
        spsum = ctx.enter_context(tc.tile_pool(name="spsum", bufs=2, space="PSUM"))
        zpsum = ctx.enter_context(tc.tile_pool(name="zpsum", bufs=3, space="PSUM"))
        ppsum = ctx.enter_context(tc.tile_pool(name="ppsum", bufs=1, space="PSUM"))
        dramp = ctx.enter_context(tc.tile_pool(name="dramp", bufs=2, space="DRAM"))

        # ---- constants ----
        bq_t = const.tile([P, NPL], F32)
        nc.sync.dma_start(bq_t[:], bq_r[:, :])
        bk_t = const.tile([P, NPL], F32)
        nc.sync.dma_start(bk_t[:], bk_r[:, :])
        bo_t = const.tile([P, DIN], F32)
        nc.sync.dma_start(bo_t[:], bo_r[:, :])
        bvb = const.tile([P, DH], F32)
        nc.sync.dma_start(bvb[:], bv_r[0:1, :].to_broadcast((P, DH)))

        # ---- X^T + weight loads, chunk-interleaved ----
        xt_t = big.tile([P, DIN, S], BF16, tag="xt")
        wk_t = wpool.tile([P, DIN, DH], BF16, tag="w", name="wk_t")
        wv_t = wpool.tile([P, DIN, DH], BF16, tag="w", name="wv_t")
        wq_t = wpool.tile([P, DIN, DH], BF16, tag="w", name="wq_t")
        for c in range(DIN):
            nc.sync.dma_start(wk_t[:, c, :], wk_r[:, c, :])
            nc.sync.dma_start(xt_t[:, c, :], xt_r[:, c, :])
            nc.sync.dma_start(wv_t[:, c, :], wv_r[:, c, :])
            nc.sync.dma_start(wq_t[:, c, :], wq_r[:, c, :])

        kt = [big.tile([P, S], BF16, tag=f"kt{p}", name=f"kt{p}")
              for p in range(NPL)]
        qt = [big.tile([P, S], BF16, tag=f"qt{p}", name=f"qt{p}")
              for p in range(NPL)]
        vt = [big.tile([P, NTOK, 130], BF16, tag=f"vt{p}", name=f"vt{p}")
              for p in range(NPL)]
        zt = [big.tile([P, S], BF16, tag=f"zt{p}", name=f"zt{p}")
              for p in range(NPL)]

        # ones columns of vt (col 64 of each head's 65-wide block)
        for pr in range(NPL):
            ones_ap = vt[pr].rearrange("p t (h c) -> p t h c", c=65)[:, :, :, 64:65]
            nc.vector.memset(ones_ap, 1.0)

        def k_proj(pr):
            for t in range(TOKC):
                ps = spsum.tile([P, 512], F32, tag="sc", name="psk")
                for c in range(DIN):
                    nc.tensor.matmul(
                        ps[:],
                        wk_t[:, c, pr * P:(pr + 1) * P],
                        xt_t[:, c, t * 512:(t + 1) * 512],
                        start=(c == 0), stop=(c == DIN - 1),
                    )
                nc.vector.tensor_scalar_add(
                    kt[pr][:, t * 512:(t + 1) * 512], ps[:], bk_t[:, pr:pr + 1]
                )

        def v_proj(g):
            # V projection for pair-group g (pairs 2g, 2g+1; 256 dims)
            prs = (2 * g, 2 * g + 1)
            for t in range(NTOK):
                ps = ppsum.tile([P, 256], F32, tag="proj", name="psv")
                for c in range(DIN):
                    nc.tensor.matmul(
                        ps[:],
                        xt_t[:, c, t * P:(t + 1) * P],
                        wv_t[:, c, g * 256:(g + 1) * 256],
                        start=(c == 0), stop=(c == DIN - 1),
                    )
                for u in range(2):
                    dst = vt[prs[u]].rearrange(
                        "p t (h c) -> p t h c", c=65)[:, t, :, 0:64]
                    src = ps[:, u * 128:(u + 1) * 128].rearrange(
                        "p (h c) -> p h c", c=64)
                    bsrc = bvb[:, (g * 256 + u * 128):
                               (g * 256 + (u + 1) * 128)
                               ].rearrange("p (h c) -> p h c", c=64)
                    nc.vector.tensor_add(dst, src, bsrc)

        def q_proj(pr):
            for t in range(QCN):
                ps = ppsum.tile([P, 512], F32, tag="proj", name="psq")
                for c in range(DIN):
                    nc.tensor.matmul(
                        ps[:],
                        wq_t[:, c, pr * P:(pr + 1) * P],
                        xt_t[:, c, t * 512:(t + 1) * 512],
                        start=(c == 0), stop=(c == DIN - 1),
                    )
                nc.vector.tensor_scalar_add(
                    qt[pr][:, t * 512:(t + 1) * 512], ps[:],
                    bq_t[:, pr:pr + 1]
                )

        def attention(pr):
            vpr = vt[pr].rearrange("p t (h c) -> p t h c", c=65)
            for qc in range(QCN):
                qsl = slice(qc * 512, (qc + 1) * 512)
                za = zpsum.tile([P, 512], F32, tag="z", name=f"za{pr}_{qc}")
                zb = zpsum.tile([P, 512], F32, tag="z", name=f"zb{pr}_{qc}")
                for kc in range(KC):
                    sq = spsum.tile([P, 2, 512], F32, tag="sc", name="sq")
                    nc.tensor.matmul(
                        sq[:, 0, :],
                        kt[pr][0:64, kc * P:(kc + 1) * P],
                        qt[pr][0:64, qsl],
                        start=True, stop=True,
                    )
                    nc.tensor.matmul(
                        sq[:, 1, :],
                        kt[pr][64:128, kc * P:(kc + 1) * P],
                        qt[pr][64:128, qsl],
                        start=True, stop=True,
                    )
                    pq = probs_pool.tile([P, 2, 512], BF16, tag="probs",
                                         name="pq")
                    nc.scalar.activation(pq[:], sq[:], EXP, scale=0.125)
                    nc.tensor.matmul(
                        za[0:65, :], vpr[:, kc, 0, :], pq[:, 0, :],
                        start=(kc == 0), stop=(kc == KC - 1),
                    )
                    nc.tensor.matmul(
                        zb[0:65, :], vpr[:, kc, 1, :], pq[:, 1, :],
                        start=(kc == 0), stop=(kc == KC - 1),
                    )
                # normalize + evict Z^T; 1/rowsum partition-broadcast
                # goes through a DRAM bounce (SBUF->SBUF bcast illegal)
                rcin = work.tile([P, 2, 512], F32, tag="rcin", name="rcin",
                                 bufs=2)
                rsc = dramp.tile([2, 512], F32, tag="rsc", name="rsc")
                nc.scalar.copy(rcin[64:65, 0, :], za[64:65, :])
                nc.vector.tensor_copy(rcin[64:65, 1, :], zb[64:65, :])
                nc.sync.dma_start(rsc[0:1, :], rcin[64:65, 0, :])
                nc.sync.dma_start(rsc[1:2, :], rcin[64:65, 1, :])
                rbr = work.tile([P, 2, 512], F32, tag="rbc", name="rbr")
                nc.sync.dma_start(rbr[0:64, 0, :],
                                  rsc[0:1, :].to_broadcast((64, 512)))
                nc.sync.dma_start(rbr[0:64, 1, :],
                                  rsc[1:2, :].to_broadcast((64, 512)))
                rb = work.tile([P, 2, 512], F32, tag="rbc", name="rb")
                nc.vector.reciprocal_approx_fast(rb[0:64, :, :],
                                                 rbr[0:64, :, :])
                nc.vector.tensor_mul(zt[pr][0:64, qsl], za[0:64, :],
                                     rb[0:64, 0, :])
                zs = work.tile([P, 512], BF16, tag="zstage", name="zs")
                nc.vector.tensor_mul(zs[0:64, :], zb[0:64, :],
                                     rb[0:64, 1, :])
                nc.sync.dma_start(zt[pr][64:128, qsl], zs[0:64, :])

        # ---- pipelined schedule ----
        k_proj(0)
        v_proj(0)
        q_proj(0)
        attention(0)
        k_proj(1)
        q_proj(1)
        attention(1)
        k_proj(2)
        k_proj(3)
        v_proj(1)
        wo_t = wpool.tile([P, NPL, D], BF16, tag="wo", name="wo_t", bufs=1)
        for c in range(NPL):
            nc.sync.dma_start(wo_t[:, c, :], wo_r[:, c, :])
        q_proj(2)
        attention(2)
        q_proj(3)
        attention(3)

        # ---- partial output projection over the 4 local pair-chunks ----
        for qc in range(QCN):
            oqsl = slice(qc * 512, (qc + 1) * 512)
            for ec in range(DIN):
                ps = spsum.tile([P, 512], F32, tag="sc", name="pso")
                for dc in range(NPL):
                    nc.tensor.matmul(
                        ps[:],
                        wo_t[:, dc, ec * P:(ec + 1) * P],
                        zt[dc][:, oqsl],
                        start=(dc == 0), stop=(dc == NPL - 1),
                    )
                st = work.tile([P, 512], F32, tag="stage", name="st", bufs=2)
                nc.vector.tensor_scalar_add(st[:], ps[:], bo_t[:, ec:ec + 1])
                nc.sync.dma_start(ot_d[ec * P:(ec + 1) * P, oqsl], st[:])

    nc.compile()
    return nc



# ---------------- host-side entry point ----------------

BF = ml_dtypes.bfloat16
_B, _S, _D, _H = 4, 2048, 1024, 16
_DH = _D // 2
_NC_CACHE = None


def _get_nc():
    global _NC_CACHE
    if _NC_CACHE is None:
        _NC_CACHE = build_attention_nc(S=_S, D=_D, DH=_DH, H=_H)
    return _NC_CACHE


def kernel(X, Wq, bq, Wk, bk, Wv, bv, Wo, bo):
    """Full-input multi-head attention on 8 TRN2 NeuronCores.

    Sharding (tensor-parallel per the head split, data-parallel over
    batch): core c handles batch c//2 and head-half c%2 (8 heads:
    column shards of Wq/Wk/Wv, row shard of Wo). Each core returns the
    partial output projection for its head dims; the host sums the two
    partials per batch while unsharding. bo rides on the hh=0 partial
    (hh=1 cores receive zeros). Inputs cast to bf16 on host; output is
    fp32 [B, S, D].
    """
    from concourse.bass_utils import run_bass_kernel_spmd

    X = np.asarray(X, dtype=np.float32)
    bq = np.asarray(bq, dtype=np.float32)
    bk = np.asarray(bk, dtype=np.float32)
    bv = np.asarray(bv, dtype=np.float32)
    bo = np.asarray(bo, dtype=np.float32)
    zeros_bo = np.zeros_like(bo)
    wqb = np.asarray(Wq, dtype=np.float32).astype(BF)
    wkb = np.asarray(Wk, dtype=np.float32).astype(BF)
    wvb = np.asarray(Wv, dtype=np.float32).astype(BF)
    wob = np.asarray(Wo, dtype=np.float32).astype(BF)

    in_maps = []
    for c in range(8):
        b, hh = c // 2, c % 2
        hsl = slice(hh * _DH, (hh + 1) * _DH)
        xtp = np.ascontiguousarray(X[b].T.astype(BF))
        in_maps.append({
            "xt": xtp,
            "wq": np.ascontiguousarray(wqb[:, hsl]),
            "wk": np.ascontiguousarray(wkb[:, hsl]),
            "wv": np.ascontiguousarray(wvb[:, hsl]),
            "wo": np.ascontiguousarray(wob[hsl, :]),
            "bq": np.ascontiguousarray(bq[hsl]),
            "bk": np.ascontiguousarray(bk[hsl]),
            "bv": np.ascontiguousarray(bv[hsl]),
            "bo": bo if hh == 0 else zeros_bo,
        })

    nc = _get_nc()
    res = run_bass_kernel_spmd(nc, in_maps, list(range(8)))

    out = np.empty((_B, _S, _D), np.float32)
    for b in range(_B):
        out[b] = (res.results[2 * b]["ot"] + res.results[2 * b + 1]["ot"]).T
    return out


# revision 25
# speedup vs baseline: 1.0280x; 1.0036x over previous
"""Bass/Tile attention kernel for TRN2 — per-core program builder.

Sharding (SPMD, core c of 8): batch b = c//2, head-half hh = c%2.
Each core projects Q/K/V for its 8 heads (4 pairs, 512 dims) over ALL
2048 tokens of its batch, runs attention for those heads, then the two
cores of a batch AllGather their Z halves and each computes a 512-wide
e-column slice of the output projection.

Inputs (per core DRAM):
  xt  : [D, S]  bf16   X[b]^T (full token set)
  wq/wk/wv : [D, DH] bf16  column slice for this core's heads
  wo  : [DH, D] bf16       ROW slice of Wo for this core's head dims
  bq/bk/bv : [DH] fp32     matching slices
  bo  : [D] fp32           full bo on hh=0 cores, zeros on hh=1
Output:
  ot  : [D, S] fp32    partial O^T = Wo_rows^T @ Z^T (+bo); the host
                       sums the two head-halves' partials per batch
                       while unsharding (no cross-core collective).

Layouts on chip (P=128 partitions):
  xt_sb[p, c, t]  = X^T[c*128+p, t]                 (bf16)
  kt[pr][p, t]    = K^T[pr*128+p, t]  (local pair pr = 2 heads)
  qt[pr][p, q]    = Q^T[pr*128+p, q]
  vt[pr][p, tk, h*65+j] = V[tk*128+p, pr*128+h*64+j] for j<64,
                          1.0 for j==64 (augmented ones col)   (bf16)
  zt[pr][p, q]    = Z^T[pr*128+p, q] (normalized)   (bf16)
  zp[pr][p, q]    = peer core's Z^T (via AllGather) (bf16)

Attention per (pair, 512-query chunk qc), PSUM chains per bank:
  for kc in range(S/128):
    scores^T[k,q] both heads -> PSUM [128, 2, 512] (2 banks) via
      row-tiled matmuls (d=64 contraction: head-lo rows 0-63,
      head-hi rows 64-127; concurrent sub-array execution).
    probs = exp(0.125*scores): one ACT instr [128, 1024] -> bf16 SBUF.
    PV per head, stationary [V_h | 1] (M=65): z PSUM [65, 512] chain
      accumulated over kc; row 64 = softmax denominator.
  reciprocal of row 64 via DRAM-bounce partition-broadcast; scale rows
  0-63 -> zt (head-hi via bf16 stage + partition-shift DMA).
O-proj partial: ot[e,q] = sum_{d in local 512} Wo[d,e] Z^T[d,q] + bo,
  accumulated over the 4 local pair-chunks, evicted with bias add
  (bo is zeros on hh=1 cores), DMA to ot.
"""

import numpy as np
import ml_dtypes
from contextlib import ExitStack

import concourse.bass as bass
import concourse.tile as tile
from concourse import bacc, mybir

F32 = mybir.dt.float32
BF16 = mybir.dt.bfloat16
P = 128


def build_attention_nc(S=2048, D=1024, DH=512, H=16):
    DK = D // H
    assert DK == 64
    NPL = DH // P             # local head pairs (4)
    KC = S // P               # k chunks (16)
    TOKC = S // 512           # 512-token col chunks for K proj (4)
    NTOK = S // P             # 128-token chunks for V proj (16)
    QCN = S // 512            # 512-query chunks (4)
    DIN = D // P              # input-dim chunks (8)

    nc = bacc.Bacc("TRN2", target_bir_lowering=False, debug=False,
                   num_devices=8)

    xt_d = nc.dram_tensor("xt", [D, S], BF16, kind="ExternalInput").ap()
    wq_d = nc.dram_tensor("wq", [D, DH], BF16, kind="ExternalInput").ap()
    wk_d = nc.dram_tensor("wk", [D, DH], BF16, kind="ExternalInput").ap()
    wv_d = nc.dram_tensor("wv", [D, DH], BF16, kind="ExternalInput").ap()
    wo_d = nc.dram_tensor("wo", [DH, D], BF16, kind="ExternalInput").ap()
    bq_d = nc.dram_tensor("bq", [DH], F32, kind="ExternalInput").ap()
    bk_d = nc.dram_tensor("bk", [DH], F32, kind="ExternalInput").ap()
    bv_d = nc.dram_tensor("bv", [DH], F32, kind="ExternalInput").ap()
    bo_d = nc.dram_tensor("bo", [D], F32, kind="ExternalInput").ap()
    ot_d = nc.dram_tensor("ot", [D, S], F32, kind="ExternalOutput").ap()

    xt_r = xt_d.rearrange("(c p) t -> p c t", p=P)
    wq_r = wq_d.rearrange("(c p) n -> p c n", p=P)
    wk_r = wk_d.rearrange("(c p) n -> p c n", p=P)
    wv_r = wv_d.rearrange("(c p) n -> p c n", p=P)
    wo_r = wo_d.rearrange("(c p) n -> p c n", p=P)  # c = NPL row-chunks
    bq_r = bq_d.rearrange("(c p) -> p c", p=P)
    bk_r = bk_d.rearrange("(c p) -> p c", p=P)
    bo_r = bo_d.rearrange("(c p) -> p c", p=P)      # c = DIN e-chunks
    bv_r = bv_d.rearrange("(a d) -> a d", a=1)

    EXP = mybir.ActivationFunctionType.Exp

    with tile.TileContext(nc) as tc, ExitStack() as ctx:
        const = ctx.enter_context(tc.tile_pool(name="const", bufs=1))
        big = ctx.enter_context(tc.tile_pool(name="big", bufs=1))
        wpool = ctx.enter_context(tc.tile_pool(name="wpool", bufs=3))
        work = ctx.enter_context(tc.tile_pool(name="work", bufs=3))
        probs_pool = ctx.enter_context(tc.tile_pool(name="probs", bufs=4))
        # PSUM budget (8 banks): scores 2x2 + z 3 + proj 1
        spsum = ctx.enter_context(tc.tile_pool(name="spsum", bufs=2, space="PSUM"))
        zpsum = ctx.enter_context(tc.tile_pool(name="zpsum", bufs=3, space="PSUM"))
        ppsum = ctx.enter_context(tc.tile_pool(name="ppsum", bufs=1, space="PSUM"))
        dramp = ctx.enter_context(tc.tile_pool(name="dramp", bufs=2, space="DRAM"))

        # ---- constants ----
        bq_t = const.tile([P, NPL], F32)
        nc.sync.dma_start(bq_t[:], bq_r[:, :])
        bk_t = const.tile([P, NPL], F32)
        nc.sync.dma_start(bk_t[:], bk_r[:, :])
        bo_t = const.tile([P, DIN], F32)
        nc.sync.dma_start(bo_t[:], bo_r[:, :])
        bvb = const.tile([P, DH], F32)
        nc.sync.dma_start(bvb[:], bv_r[0:1, :].to_broadcast((P, DH)))

        # ---- X^T + weight loads, chunk-interleaved ----
        xt_t = big.tile([P, DIN, S], BF16, tag="xt")
        wk_t = wpool.tile([P, DIN, DH], BF16, tag="w", name="wk_t")
        wv_t = wpool.tile([P, DIN, DH], BF16, tag="w", name="wv_t")
        wq_t = wpool.tile([P, DIN, DH], BF16, tag="w", name="wq_t")
        for c in range(DIN):
            nc.sync.dma_start(wk_t[:, c, :], wk_r[:, c, :])
            nc.sync.dma_start(xt_t[:, c, :], xt_r[:, c, :])
            nc.sync.dma_start(wv_t[:, c, :], wv_r[:, c, :])
            nc.sync.dma_start(wq_t[:, c, :], wq_r[:, c, :])

        kt = [big.tile([P, S], BF16, tag=f"kt{p}", name=f"kt{p}")
              for p in range(NPL)]
        qt = [big.tile([P, S], BF16, tag=f"qt{p}", name=f"qt{p}")
              for p in range(NPL)]
        vt = [big.tile([P, NTOK, 130], BF16, tag=f"vt{p}", name=f"vt{p}")
              for p in range(NPL)]
        zt = [big.tile([P, S], BF16, tag=f"zt{p}", name=f"zt{p}")
              for p in range(NPL)]

        # ones columns of vt (col 64 of each head's 65-wide block)
        for pr in range(NPL):
            ones_ap = vt[pr].rearrange("p t (h c) -> p t h c", c=65)[:, :, :, 64:65]
            nc.vector.memset(ones_ap, 1.0)

        def k_proj(pr):
            for t in range(TOKC):
                ps = ppsum.tile([P, 512], F32, tag="proj", name="psk")
                for c in range(DIN):
                    nc.tensor.matmul(
                        ps[:],
                        wk_t[:, c, pr * P:(pr + 1) * P],
                        xt_t[:, c, t * 512:(t + 1) * 512],
                        start=(c == 0), stop=(c == DIN - 1),
                    )
                nc.vector.tensor_scalar_add(
                    kt[pr][:, t * 512:(t + 1) * 512], ps[:], bk_t[:, pr:pr + 1]
                )

        def v_proj(g):
            # V projection for pair-group g (pairs 2g, 2g+1; 256 dims)
            prs = (2 * g, 2 * g + 1)
            for t in range(NTOK):
                ps = ppsum.tile([P, 256], F32, tag="proj", name="psv")
                for c in range(DIN):
                    nc.tensor.matmul(
                        ps[:],
                        xt_t[:, c, t * P:(t + 1) * P],
                        wv_t[:, c, g * 256:(g + 1) * 256],
                        start=(c == 0), stop=(c == DIN - 1),
                    )
                for u in range(2):
                    for h in range(2):
                        # contiguous 64-col slices (strided 3-AP adds are
                        # ~2x slower on DVE)
                        nc.vector.tensor_add(
                            vt[prs[u]][:, t, h * 65:h * 65 + 64],
                            ps[:, u * 128 + h * 64:u * 128 + (h + 1) * 64],
                            bvb[:, g * 256 + u * 128 + h * 64:
                                g * 256 + u * 128 + (h + 1) * 64],
                        )

        def q_proj(pr):
            for t in range(QCN):
                ps = ppsum.tile([P, 512], F32, tag="proj", name="psq")
                for c in range(DIN):
                    nc.tensor.matmul(
                        ps[:],
                        wq_t[:, c, pr * P:(pr + 1) * P],
                        xt_t[:, c, t * 512:(t + 1) * 512],
                        start=(c == 0), stop=(c == DIN - 1),
                    )
                nc.vector.tensor_scalar_add(
                    qt[pr][:, t * 512:(t + 1) * 512], ps[:],
                    bq_t[:, pr:pr + 1]
                )

        def attention(pr):
            vpr = vt[pr].rearrange("p t (h c) -> p t h c", c=65)
            for qc in range(QCN):
                qsl = slice(qc * 512, (qc + 1) * 512)
                za = zpsum.tile([P, 512], F32, tag="z", name=f"za{pr}_{qc}")
                zb = zpsum.tile([P, 512], F32, tag="z", name=f"zb{pr}_{qc}")
                for kc in range(KC):
                    sq = spsum.tile([P, 2, 512], F32, tag="sc", name="sq")
                    nc.tensor.matmul(
                        sq[:, 0, :],
                        kt[pr][0:64, kc * P:(kc + 1) * P],
                        qt[pr][0:64, qsl],
                        start=True, stop=True,
                    )
                    nc.tensor.matmul(
                        sq[:, 1, :],
                        kt[pr][64:128, kc * P:(kc + 1) * P],
                        qt[pr][64:128, qsl],
                        start=True, stop=True,
                    )
                    pq = probs_pool.tile([P, 2, 512], BF16, tag="probs",
                                         name="pq")
                    nc.scalar.activation(pq[:], sq[:], EXP, scale=0.125)
                    nc.tensor.matmul(
                        za[0:65, :], vpr[:, kc, 0, :], pq[:, 0, :],
                        start=(kc == 0), stop=(kc == KC - 1),
                    )
                    nc.tensor.matmul(
                        zb[0:65, :], vpr[:, kc, 1, :], pq[:, 1, :],
                        start=(kc == 0), stop=(kc == KC - 1),
                    )
                # normalize + evict Z^T; 1/rowsum partition-broadcast
                # goes through a DRAM bounce (SBUF->SBUF bcast illegal)
                rcin = work.tile([P, 2, 512], F32, tag="rcin", name="rcin",
                                 bufs=2)
                rsc = dramp.tile([2, 512], F32, tag="rsc", name="rsc")
                nc.scalar.copy(rcin[64:65, 0, :], za[64:65, :])
                nc.vector.tensor_copy(rcin[64:65, 1, :], zb[64:65, :])
                nc.sync.dma_start(rsc[0:1, :], rcin[64:65, 0, :])
                nc.sync.dma_start(rsc[1:2, :], rcin[64:65, 1, :])
                rbr = work.tile([P, 2, 512], F32, tag="rbc", name="rbr")
                nc.sync.dma_start(rbr[0:64, 0, :],
                                  rsc[0:1, :].to_broadcast((64, 512)))
                nc.sync.dma_start(rbr[0:64, 1, :],
                                  rsc[1:2, :].to_broadcast((64, 512)))
                rb = work.tile([P, 2, 512], F32, tag="rbc", name="rb")
                nc.vector.reciprocal_approx_fast(rb[0:64, :, :],
                                                 rbr[0:64, :, :])
                nc.vector.tensor_mul(zt[pr][0:64, qsl], za[0:64, :],
                                     rb[0:64, 0, :])
                zs = work.tile([P, 512], BF16, tag="zstage", name="zs")
                nc.vector.tensor_mul(zs[0:64, :], zb[0:64, :],
                                     rb[0:64, 1, :])
                nc.sync.dma_start(zt[pr][64:128, qsl], zs[0:64, :])

        # ---- pipelined schedule ----
        k_proj(0)
        v_proj(0)
        q_proj(0)
        attention(0)
        k_proj(1)
        q_proj(1)
        attention(1)
        k_proj(2)
        k_proj(3)
        v_proj(1)
        wo_t = wpool.tile([P, NPL, D], BF16, tag="wo", name="wo_t", bufs=1)
        for c in range(NPL):
            nc.sync.dma_start(wo_t[:, c, :], wo_r[:, c, :])
        q_proj(2)
        attention(2)
        q_proj(3)
        attention(3)

        # ---- partial output projection over the 4 local pair-chunks ----
        for qc in range(QCN):
            oqsl = slice(qc * 512, (qc + 1) * 512)
            for ec in range(DIN):
                ps = ppsum.tile([P, 512], F32, tag="proj", name="pso")
                for dc in range(NPL):
                    nc.tensor.matmul(
                        ps[:],
                        wo_t[:, dc, ec * P:(ec + 1) * P],
                        zt[dc][:, oqsl],
                        start=(dc == 0), stop=(dc == NPL - 1),
                    )
                st = work.tile([P, 512], F32, tag="stage", name="st", bufs=2)
                nc.vector.tensor_scalar_add(st[:], ps[:], bo_t[:, ec:ec + 1])
                nc.sync.dma_start(ot_d[ec * P:(ec + 1) * P, oqsl], st[:])

    nc.compile()
    return nc



# ---------------- host-side entry point ----------------

BF = ml_dtypes.bfloat16
_B, _S, _D, _H = 4, 2048, 1024, 16
_DH = _D // 2
_NC_CACHE = None


def _get_nc():
    global _NC_CACHE
    if _NC_CACHE is None:
        _NC_CACHE = build_attention_nc(S=_S, D=_D, DH=_DH, H=_H)
    return _NC_CACHE


def kernel(X, Wq, bq, Wk, bk, Wv, bv, Wo, bo):
    """Full-input multi-head attention on 8 TRN2 NeuronCores.

    Sharding (tensor-parallel per the head split, data-parallel over
    batch): core c handles batch c//2 and head-half c%2 (8 heads:
    column shards of Wq/Wk/Wv, row shard of Wo). Each core returns the
    partial output projection for its head dims; the host sums the two
    partials per batch while unsharding. bo rides on the hh=0 partial
    (hh=1 cores receive zeros). Inputs cast to bf16 on host; output is
    fp32 [B, S, D].
    """
    from concourse.bass_utils import run_bass_kernel_spmd

    X = np.asarray(X, dtype=np.float32)
    bq = np.asarray(bq, dtype=np.float32)
    bk = np.asarray(bk, dtype=np.float32)
    bv = np.asarray(bv, dtype=np.float32)
    bo = np.asarray(bo, dtype=np.float32)
    zeros_bo = np.zeros_like(bo)
    wqb = np.asarray(Wq, dtype=np.float32).astype(BF)
    wkb = np.asarray(Wk, dtype=np.float32).astype(BF)
    wvb = np.asarray(Wv, dtype=np.float32).astype(BF)
    wob = np.asarray(Wo, dtype=np.float32).astype(BF)

    in_maps = []
    for c in range(8):
        b, hh = c // 2, c % 2
        hsl = slice(hh * _DH, (hh + 1) * _DH)
        xtp = np.ascontiguousarray(X[b].T.astype(BF))
        in_maps.append({
            "xt": xtp,
            "wq": np.ascontiguousarray(wqb[:, hsl]),
            "wk": np.ascontiguousarray(wkb[:, hsl]),
            "wv": np.ascontiguousarray(wvb[:, hsl]),
            "wo": np.ascontiguousarray(wob[hsl, :]),
            "bq": np.ascontiguousarray(bq[hsl]),
            "bk": np.ascontiguousarray(bk[hsl]),
            "bv": np.ascontiguousarray(bv[hsl]),
            "bo": bo if hh == 0 else zeros_bo,
        })

    nc = _get_nc()
    res = run_bass_kernel_spmd(nc, in_maps, list(range(8)))

    out = np.empty((_B, _S, _D), np.float32)
    for b in range(_B):
        out[b] = (res.results[2 * b]["ot"] + res.results[2 * b + 1]["ot"]).T
    return out


# revision 26
# speedup vs baseline: 1.0603x; 1.0314x over previous
"""Bass/Tile attention kernel for TRN2 — per-core program builder.

Sharding (SPMD, core c of 8): batch b = c//2, head-half hh = c%2
(tensor-parallel head split per the classic Megatron decomposition:
column shards of Wq/Wk/Wv, row shard of Wo; data-parallel over batch).
Each core projects Q/K/V for its 8 heads (4 pairs, 512 dims) over ALL
2048 tokens of its batch, runs attention for those heads, and emits
the PARTIAL output projection over its 512 Z-dims; the host sums the
two partials per batch while unsharding (no cross-core collective).

Inputs (per core DRAM):
  xt  : [D, S]  bf16   X[b]^T (full token set)
  wq/wk/wv : [D, DH] bf16  column slice for this core's heads
  wo  : [DH, D] bf16       row slice of Wo for this core's head dims
  bq/bk/bv : [DH] fp32     matching slices
  bo  : [D] fp32           full bo on hh=0 cores, zeros on hh=1
Output:
  ot  : [D, S] fp32    partial O^T (host transposes and sums pairs).

Layouts on chip (P=128 partitions):
  xt_sb[p, c, t]  = X^T[c*128+p, t]                 (bf16)
  kt[pr][p, t]    = K^T[pr*128+p, t]   (local pair pr = 2 heads)
  qt[pr][p, q]    = Q^T[pr*128+p, q]
  vt[pr][p, tk, h*65+j] = V[tk*128+p, pr*128+h*64+j] for j<64,
                          1.0 for j==64 (augmented ones col)   (bf16)
  zt[pr][p, q]    = Z^T[pr*128+p, q] (normalized)   (bf16)

Attention per (pair, 512-query chunk qc), one PSUM accumulation chain
per bank (zero-region rule: a bank holds one chain at a time):
  for kc in range(S/128):
    scores^T[k,q] both heads -> PSUM tile [128, 2, 512] (2 banks), via
      row-packed matmuls (contraction d=64: head-lo array rows 0-63,
      head-hi rows 64-127).
    probs = exp(0.125*scores): one ACT instruction [128, 1024] -> bf16.
    PV: per head, stationary [V_h | 1] (M=65): z_h PSUM [65, 512] chain
      accumulated over kc; row 64 = softmax denominator (rowsum).
  reciprocal of row 64, DRAM-bounce partition-broadcast to [64, 512],
  multiply rows 0-63 -> zt (head-hi goes via a bf16 stage +
  partition-shift DMA into zt[pr][64:128]).
O-proj partial: ot[e,q] = sum_{d in local 512} Wo[d,e] Z^T[d,q] + bo,
accumulated over the 4 local pair-chunks, evicted with bias add, DMA
to ot.
"""

import numpy as np
import ml_dtypes
from contextlib import ExitStack

import concourse.bass as bass
import concourse.tile as tile
from concourse import bacc, mybir

F32 = mybir.dt.float32
BF16 = mybir.dt.bfloat16
P = 128


def build_attention_nc(S_full=2048, D=1024, DH=512, H=16):
    DK = D // H
    assert DK == 64
    NPAIR = DH // P           # local head pairs (4; 128 dims each)
    NQUAD = NPAIR // 2        # quads of 2 pairs (2)
    KC = S_full // P          # k chunks
    TOKC = S_full // 512      # 512-token col chunks for K proj
    NTOK = S_full // P        # 128-token chunks for V proj
    QCN = S_full // 512       # 512-query chunks (full token set)
    DIN = D // P              # input-dim chunks

    nc = bacc.Bacc("TRN2", target_bir_lowering=False, debug=False)

    xt_d = nc.dram_tensor("xt", [D, S_full], BF16, kind="ExternalInput").ap()
    wq_d = nc.dram_tensor("wq", [D, DH], BF16, kind="ExternalInput").ap()
    wk_d = nc.dram_tensor("wk", [D, DH], BF16, kind="ExternalInput").ap()
    wv_d = nc.dram_tensor("wv", [D, DH], BF16, kind="ExternalInput").ap()
    wo_d = nc.dram_tensor("wo", [DH, D], BF16, kind="ExternalInput").ap()
    bq_d = nc.dram_tensor("bq", [DH], F32, kind="ExternalInput").ap()
    bk_d = nc.dram_tensor("bk", [DH], F32, kind="ExternalInput").ap()
    bv_d = nc.dram_tensor("bv", [DH], F32, kind="ExternalInput").ap()
    bo_d = nc.dram_tensor("bo", [D], F32, kind="ExternalInput").ap()
    ot_d = nc.dram_tensor("ot", [D, S_full], F32, kind="ExternalOutput").ap()

    xt_r = xt_d.rearrange("(c p) t -> p c t", p=P)
    wq_r = wq_d.rearrange("(c p) n -> p c n", p=P)
    wk_r = wk_d.rearrange("(c p) n -> p c n", p=P)
    wv_r = wv_d.rearrange("(c p) n -> p c n", p=P)
    wo_r = wo_d.rearrange("(c p) n -> p c n", p=P)  # c = NPAIR row-chunks
    bq_r = bq_d.rearrange("(c p) -> p c", p=P)
    bk_r = bk_d.rearrange("(c p) -> p c", p=P)
    bo_r = bo_d.rearrange("(c p) -> p c", p=P)      # c = DIN e-chunks
    bv_r = bv_d.rearrange("(a d) -> a d", a=1)

    EXP = mybir.ActivationFunctionType.Exp

    with tile.TileContext(nc) as tc, ExitStack() as ctx:
        const = ctx.enter_context(tc.tile_pool(name="const", bufs=1))
        big = ctx.enter_context(tc.tile_pool(name="big", bufs=1))
        wpool = ctx.enter_context(tc.tile_pool(name="wpool", bufs=2))
        work = ctx.enter_context(tc.tile_pool(name="work", bufs=3))
        probs_pool = ctx.enter_context(tc.tile_pool(name="probs", bufs=6))
        # PSUM budget (8 banks): scores 2x2 + z 3 + proj 1
        spsum = ctx.enter_context(tc.tile_pool(name="spsum", bufs=2, space="PSUM"))
        zpsum = ctx.enter_context(tc.tile_pool(name="zpsum", bufs=3, space="PSUM"))
        ppsum = ctx.enter_context(tc.tile_pool(name="ppsum", bufs=1, space="PSUM"))
        dramp = ctx.enter_context(tc.tile_pool(name="dramp", bufs=2, space="DRAM"))

        # ---- constants ----
        bq_t = const.tile([P, NPAIR], F32)
        nc.sync.dma_start(bq_t[:], bq_r[:, :])
        bk_t = const.tile([P, NPAIR], F32)
        nc.sync.dma_start(bk_t[:], bk_r[:, :])
        bo_t = const.tile([P, DIN], F32)
        nc.sync.dma_start(bo_t[:], bo_r[:, :])
        bvb = const.tile([P, DH], F32)
        nc.sync.dma_start(bvb[:], bv_r[0:1, :].to_broadcast((P, DH)))

        # ---- X^T + Wk loads, chunk-interleaved so K-proj starts early ----
        xt_t = big.tile([P, DIN, S_full], BF16, tag="xt")
        wk_t = wpool.tile([P, DIN, DH], BF16, tag="w", name="wk_t")
        for c in range(DIN):
            nc.sync.dma_start(wk_t[:, c, :], wk_r[:, c, :])
            nc.sync.dma_start(xt_t[:, c, :], xt_r[:, c, :])

        kt = [big.tile([P, S_full], BF16, tag=f"kt{p}", name=f"kt{p}")
              for p in range(NPAIR)]
        qt = [big.tile([P, S_full], BF16, tag=f"qt{p}", name=f"qt{p}")
              for p in range(NPAIR)]
        # per-pair augmented V: 2 heads x (64 data + 1 ones col)
        vt = [big.tile([P, NTOK, 130], BF16, tag=f"vt{p}", name=f"vt{p}")
              for p in range(NPAIR)]
        zt = [big.tile([P, S_full], BF16, tag=f"zt{p}", name=f"zt{p}")
              for p in range(NPAIR)]

        # ones columns of vt (col 64 of each head's 65-wide block)
        for pr in range(NPAIR):
            ones_ap = vt[pr].rearrange("p t (h c) -> p t h c", c=65)[:, :, :, 64:65]
            nc.vector.memset(ones_ap, 1.0)

        # ---- weight loads (double-buffered slots) ----
        def load_w(w_r, nm, nchunk=DIN):
            w_t = wpool.tile([P, DIN, DH], BF16, tag="w", name=nm)
            for c in range(nchunk):
                nc.sync.dma_start(w_t[:, c, :], w_r[:, c, :])
            return w_t

        wv_t = load_w(wv_r, "wv_t")

        # ---- K^T projection for all local pairs (frees wk's slot) ----
        for pr in range(NPAIR):
            for t in range(TOKC):
                ps = spsum.tile([P, 512], F32, tag="sc", name="psk")
                for c in range(DIN):
                    nc.tensor.matmul(
                        ps[:],
                        wk_t[:, c, pr * P:(pr + 1) * P],
                        xt_t[:, c, t * 512:(t + 1) * 512],
                        start=(c == 0), stop=(c == DIN - 1),
                    )
                nc.vector.tensor_scalar_add(
                    kt[pr][:, t * 512:(t + 1) * 512], ps[:], bk_t[:, pr:pr + 1]
                )
        wq_t = load_w(wq_r, "wq_t")

        # ---- V/Q projections + attention, per quad / pair ----
        for g in range(NQUAD):
            prs = (2 * g, 2 * g + 1)
            # V projection for the quad's 256 columns; evict 128 cols
            # into each pair tile (contiguous per-head slices)
            for t in range(NTOK):
                ps = ppsum.tile([P, 256], F32, tag="proj", name="psv")
                for c in range(DIN):
                    nc.tensor.matmul(
                        ps[:],
                        xt_t[:, c, t * P:(t + 1) * P],
                        wv_t[:, c, g * 256:(g + 1) * 256],
                        start=(c == 0), stop=(c == DIN - 1),
                    )
                for u in range(2):
                    for h in range(2):
                        nc.vector.tensor_add(
                            vt[prs[u]][:, t, h * 65:h * 65 + 64],
                            ps[:, u * 128 + h * 64:u * 128 + (h + 1) * 64],
                            bvb[:, g * 256 + u * 128 + h * 64:
                                g * 256 + u * 128 + (h + 1) * 64],
                        )
            # Q^T projection for the quad's two pairs (all q chunks)
            for pr in prs:
                for t in range(QCN):
                    ps = ppsum.tile([P, 512], F32, tag="proj",
                                    name="psq")
                    for c in range(DIN):
                        nc.tensor.matmul(
                            ps[:],
                            wq_t[:, c, pr * P:(pr + 1) * P],
                            xt_t[:, c, t * 512:(t + 1) * 512],
                            start=(c == 0), stop=(c == DIN - 1),
                        )
                    nc.vector.tensor_scalar_add(
                        qt[pr][:, t * 512:(t + 1) * 512], ps[:],
                        bq_t[:, pr:pr + 1]
                    )

            # ---- attention for each pair of the quad ----
            for pr in prs:
                vpr = vt[pr].rearrange("p t (h c) -> p t h c", c=65)
                for qc in range(QCN):
                    qsl = slice(qc * 512, (qc + 1) * 512)
                    za = zpsum.tile([P, 512], F32, tag="z", name=f"za{pr}_{qc}")
                    zb = zpsum.tile([P, 512], F32, tag="z", name=f"zb{pr}_{qc}")
                    for kc in range(KC):
                        sq = spsum.tile([P, 2, 512], F32, tag="sc", name="sq")
                        nc.tensor.matmul(
                            sq[:, 0, :],
                            kt[pr][0:64, kc * P:(kc + 1) * P],
                            qt[pr][0:64, qsl],
                            start=True, stop=True,
                        )
                        nc.tensor.matmul(
                            sq[:, 1, :],
                            kt[pr][64:128, kc * P:(kc + 1) * P],
                            qt[pr][64:128, qsl],
                            start=True, stop=True,
                        )
                        pq = probs_pool.tile([P, 2, 512], BF16, tag="probs",
                                             name="pq")
                        nc.scalar.activation(pq[:], sq[:], EXP, scale=0.125)
                        nc.tensor.matmul(
                            za[0:65, :], vpr[:, kc, 0, :], pq[:, 0, :],
                            start=(kc == 0), stop=(kc == KC - 1),
                        )
                        nc.tensor.matmul(
                            zb[0:65, :], vpr[:, kc, 1, :], pq[:, 1, :],
                            start=(kc == 0), stop=(kc == KC - 1),
                        )
                    # normalize + evict Z^T; 1/rowsum partition-broadcast
                    # goes through a DRAM bounce (SBUF->SBUF bcast illegal)
                    rcin = work.tile([P, 2, 512], F32, tag="rcin", name="rcin", bufs=2)
                    rsc = dramp.tile([2, 512], F32, tag="rsc", name="rsc")
                    # Stage rowsum rows to SBUF via ScalarE (the approx-recip
                    # custom-DVE op misreads PSUM on HW), DRAM-bounce them to
                    # a partition-broadcast tile, then take the reciprocal
                    # there (the custom op also requires base partition 0).
                    nc.scalar.copy(rcin[64:65, 0, :], za[64:65, :])
                    nc.vector.tensor_copy(rcin[64:65, 1, :], zb[64:65, :])
                    nc.sync.dma_start(rsc[0:1, :], rcin[64:65, 0, :])
                    nc.sync.dma_start(rsc[1:2, :], rcin[64:65, 1, :])
                    rbr = work.tile([P, 2, 512], F32, tag="rbc", name="rbr")
                    nc.sync.dma_start(rbr[0:64, 0, :],
                                      rsc[0:1, :].to_broadcast((64, 512)))
                    nc.sync.dma_start(rbr[0:64, 1, :],
                                      rsc[1:2, :].to_broadcast((64, 512)))
                    rb = work.tile([P, 2, 512], F32, tag="rbc", name="rb")
                    nc.vector.reciprocal_approx_fast(rb[0:64, :, :],
                                                     rbr[0:64, :, :])
                    nc.vector.tensor_mul(zt[pr][0:64, qsl], za[0:64, :],
                                         rb[0:64, 0, :])
                    zs = work.tile([P, 512], BF16, tag="zstage", name="zs")
                    nc.vector.tensor_mul(zs[0:64, :], zb[0:64, :],
                                         rb[0:64, 1, :])
                    nc.sync.dma_start(zt[pr][64:128, qsl], zs[0:64, :])

        # ---- partial output projection over the 4 local pair-chunks ----
        wo_t = wpool.tile([P, NPAIR, D], BF16, tag="w", name="wo_t")
        for c in range(NPAIR):
            nc.sync.dma_start(wo_t[:, c, :], wo_r[:, c, :])
        for qc in range(QCN):
            oqsl = slice(qc * 512, (qc + 1) * 512)
            for ec in range(DIN):
                ps = spsum.tile([P, 512], F32, tag="sc", name="pso")
                for dc in range(NPAIR):
                    nc.tensor.matmul(
                        ps[:],
                        wo_t[:, dc, ec * P:(ec + 1) * P],
                        zt[dc][:, oqsl],
                        start=(dc == 0), stop=(dc == NPAIR - 1),
                    )
                st = work.tile([P, 512], F32, tag="stage", name="st", bufs=2)
                nc.vector.tensor_scalar_add(st[:], ps[:], bo_t[:, ec:ec + 1])
                nc.sync.dma_start(ot_d[ec * P:(ec + 1) * P, oqsl], st[:])

    nc.compile()
    return nc



# ---------------- host-side entry point ----------------

BF = ml_dtypes.bfloat16
_B, _S, _D, _H = 4, 2048, 1024, 16
_DH = _D // 2
_NC_CACHE = None


def _get_nc():
    global _NC_CACHE
    if _NC_CACHE is None:
        _NC_CACHE = build_attention_nc(S_full=_S, D=_D, DH=_DH, H=_H)
    return _NC_CACHE


def kernel(X, Wq, bq, Wk, bk, Wv, bv, Wo, bo):
    """Full-input multi-head attention on 8 TRN2 NeuronCores.

    Sharding (per the tensor-parallel head split): core c handles batch
    c//2 and head-half c%2 (column shards of Wq/Wk/Wv, row shard of
    Wo). Each core returns the partial output projection for its head
    dims; the host sums the two partials per batch while unsharding.
    bo rides on the hh=0 partial (hh=1 cores receive zeros). Inputs
    cast to bf16 on host; output is fp32 [B, S, D].
    """
    from concourse.bass_utils import run_bass_kernel_spmd

    X = np.asarray(X, dtype=np.float32)
    bq = np.asarray(bq, dtype=np.float32)
    bk = np.asarray(bk, dtype=np.float32)
    bv = np.asarray(bv, dtype=np.float32)
    bo = np.asarray(bo, dtype=np.float32)
    zeros_bo = np.zeros_like(bo)
    wqb = np.asarray(Wq, dtype=np.float32).astype(BF)
    wkb = np.asarray(Wk, dtype=np.float32).astype(BF)
    wvb = np.asarray(Wv, dtype=np.float32).astype(BF)
    wob = np.asarray(Wo, dtype=np.float32).astype(BF)

    in_maps = []
    for c in range(8):
        b, hh = c // 2, c % 2
        hsl = slice(hh * _DH, (hh + 1) * _DH)
        xtp = np.ascontiguousarray(X[b].T.astype(BF))
        in_maps.append({
            "xt": xtp,
            "wq": np.ascontiguousarray(wqb[:, hsl]),
            "wk": np.ascontiguousarray(wkb[:, hsl]),
            "wv": np.ascontiguousarray(wvb[:, hsl]),
            "wo": np.ascontiguousarray(wob[hsl, :]),
            "bq": np.ascontiguousarray(bq[hsl]),
            "bk": np.ascontiguousarray(bk[hsl]),
            "bv": np.ascontiguousarray(bv[hsl]),
            "bo": bo if hh == 0 else zeros_bo,
        })

    nc = _get_nc()
    res = run_bass_kernel_spmd(nc, in_maps, list(range(8)))

    out = np.empty((_B, _S, _D), np.float32)
    for b in range(_B):
        out[b] = (res.results[2 * b]["ot"] + res.results[2 * b + 1]["ot"]).T
    return out


# revision 27
# speedup vs baseline: 1.3007x; 1.2267x over previous
"""Bass/Tile attention kernel for TRN2 — per-core program builder.

Per-core work (SPMD, core c of 8): batch b = c//2, query-half = c%2.
Inputs (per core DRAM):
  xt  : [D, S]  bf16   X[b]^T with token columns permuted so cols 0..SQ-1
                       are this core's query tokens (K/V use all S tokens;
                       token order is irrelevant for softmax/PV).
  wq/wk/wv/wo : [D, D] bf16 (natural [d_in, d_out] / [d, e] layout)
  bq/bk/bv/bo : [D] fp32
Output:
  ot  : [D, SQ] fp32   O^T for this core's query half (host transposes).

Layouts on chip (P=128 partitions):
  xt_sb[p, c, t]  = X^T[c*128+p, t]                 (bf16)
  kt[pair][p, t]  = K^T[pair*128+p, t]              (bf16)  pair = 2 heads
  qt[pair][p, q]  = Q^T[pair*128+p, q]              (bf16)
  vt[pair][p, tk, h*65+j] = V[tk*128+p, pair*128+h*64+j] for j<64,
                            1.0 for j==64 (augmented ones col)   (bf16)
  zt[pair][p, q]  = Z^T[pair*128+p, q] (normalized) (bf16)

Attention per (pair, 512-query chunk qc), one PSUM accumulation chain per
bank (zero-region rule: a bank may hold only one chain at a time):
  for kc in range(S/128):
    scores^T[k,q] both heads -> PSUM tile [128, 2, 512] (2 banks), via
      row-packed single-shot matmuls (contraction d=64: head-lo array rows
      0-63, head-hi rows 64-127).
    probs = exp(0.125*scores): one ACT instruction [128, 1024] -> bf16 SBUF.
    PV: per head, stationary [V_h | 1] (M=65): z_h PSUM [65, 512] chain
      accumulated over kc; row 64 = softmax denominator (rowsum).
  reciprocal of row 64, DRAM-bounce partition-broadcast to [64, 512],
  multiply rows 0-63 -> zt (head-hi goes via a bf16 stage + partition-shift
  DMA into zt[pair][64:128]).
O-proj: O^T[e,q] = sum_d Wo[d,e] Z^T[d,q] + bo, accumulated over 8
pair-chunks of d, evicted with bias add, DMA to ot.
"""

import numpy as np
import ml_dtypes
from contextlib import ExitStack

import concourse.bass as bass
import concourse.tile as tile
from concourse import bacc, mybir

F32 = mybir.dt.float32
BF16 = mybir.dt.bfloat16
P = 128


def build_attention_nc(S_full=2048, SQ=1024, D=1024, H=16):
    DK = D // H
    assert DK == 64
    NPAIR = D // P            # head pairs (128 dims each)
    NQUAD = NPAIR // 2
    KC = S_full // P          # k chunks
    TOKC = S_full // 512      # 512-token col chunks for K proj
    NTOK = S_full // P        # 128-token chunks for V proj
    QCN = SQ // 512           # 512-query chunks
    DIN = D // P              # input-dim chunks

    nc = bacc.Bacc("TRN2", target_bir_lowering=False, debug=False)

    xt_d = nc.dram_tensor("xt", [D, S_full], BF16, kind="ExternalInput").ap()
    wq_d = nc.dram_tensor("wq", [D, D], BF16, kind="ExternalInput").ap()
    wk_d = nc.dram_tensor("wk", [D, D], BF16, kind="ExternalInput").ap()
    wv_d = nc.dram_tensor("wv", [D, D], BF16, kind="ExternalInput").ap()
    wo_d = nc.dram_tensor("wo", [D, D], BF16, kind="ExternalInput").ap()
    bq_d = nc.dram_tensor("bq", [D], F32, kind="ExternalInput").ap()
    bk_d = nc.dram_tensor("bk", [D], F32, kind="ExternalInput").ap()
    bv_d = nc.dram_tensor("bv", [D], F32, kind="ExternalInput").ap()
    bo_d = nc.dram_tensor("bo", [D], F32, kind="ExternalInput").ap()
    ot_d = nc.dram_tensor("ot", [D, SQ], F32, kind="ExternalOutput").ap()

    xt_r = xt_d.rearrange("(c p) t -> p c t", p=P)
    wq_r = wq_d.rearrange("(c p) n -> p c n", p=P)
    wk_r = wk_d.rearrange("(c p) n -> p c n", p=P)
    wv_r = wv_d.rearrange("(c p) n -> p c n", p=P)
    wo_r = wo_d.rearrange("(c p) n -> p c n", p=P)
    bq_r = bq_d.rearrange("(c p) -> p c", p=P)
    bk_r = bk_d.rearrange("(c p) -> p c", p=P)
    bo_r = bo_d.rearrange("(c p) -> p c", p=P)
    bv_r = bv_d.rearrange("(a d) -> a d", a=1)

    EXP = mybir.ActivationFunctionType.Exp

    with tile.TileContext(nc) as tc, ExitStack() as ctx:
        const = ctx.enter_context(tc.tile_pool(name="const", bufs=1))
        big = ctx.enter_context(tc.tile_pool(name="big", bufs=1))
        wpool = ctx.enter_context(tc.tile_pool(name="wpool", bufs=2))
        work = ctx.enter_context(tc.tile_pool(name="work", bufs=3))
        probs_pool = ctx.enter_context(tc.tile_pool(name="probs", bufs=6))
        # PSUM budget (8 banks): scores 2x2 + z 3 + proj 1
        spsum = ctx.enter_context(tc.tile_pool(name="spsum", bufs=2, space="PSUM"))
        zpsum = ctx.enter_context(tc.tile_pool(name="zpsum", bufs=3, space="PSUM"))
        ppsum = ctx.enter_context(tc.tile_pool(name="ppsum", bufs=1, space="PSUM"))
        dramp = ctx.enter_context(tc.tile_pool(name="dramp", bufs=2, space="DRAM"))

        # ---- constants ----
        bq_t = const.tile([P, DIN], F32)
        nc.sync.dma_start(bq_t[:], bq_r[:, :])
        bk_t = const.tile([P, DIN], F32)
        nc.sync.dma_start(bk_t[:], bk_r[:, :])
        bo_t = const.tile([P, DIN], F32)
        nc.sync.dma_start(bo_t[:], bo_r[:, :])
        bvb = const.tile([P, D], F32)
        nc.sync.dma_start(bvb[:], bv_r[0:1, :].to_broadcast((P, D)))

        # ---- X^T + Wk loads, chunk-interleaved so K-proj starts early ----
        xt_t = big.tile([P, DIN, S_full], BF16, tag="xt")
        wk_t = wpool.tile([P, DIN, D], BF16, tag="w", name="wk_t")
        for c in range(DIN):
            nc.sync.dma_start(wk_t[:, c, :], wk_r[:, c, :])
            nc.sync.dma_start(xt_t[:, c, :], xt_r[:, c, :])

        kt = [big.tile([P, S_full], BF16, tag=f"kt{p}", name=f"kt{p}")
              for p in range(NPAIR)]
        qt = [big.tile([P, SQ], BF16, tag=f"qt{p}", name=f"qt{p}")
              for p in range(NPAIR)]
        # per-pair augmented V: 2 heads x (64 data + 1 ones col)
        vt = [big.tile([P, NTOK, 130], BF16, tag=f"vt{p}", name=f"vt{p}")
              for p in range(NPAIR)]
        zt = [big.tile([P, SQ], BF16, tag=f"zt{p}", name=f"zt{p}")
              for p in range(NPAIR)]

        # ones columns of vt (col 64 of each head's 65-wide block)
        for pr in range(NPAIR):
            ones_ap = vt[pr].rearrange("p t (h c) -> p t h c", c=65)[:, :, :, 64:65]
            nc.vector.memset(ones_ap, 1.0)

        # ---- weight loads (double-buffered slots) ----
        def load_w(w_r, nm):
            w_t = wpool.tile([P, DIN, D], BF16, tag="w", name=nm)
            for c in range(DIN):
                nc.sync.dma_start(w_t[:, c, :], w_r[:, c, :])
            return w_t

        wv_t = load_w(wv_r, "wv_t")

        # ---- K^T projection for all pairs (frees wk's slot for wq) ----
        for pr in range(NPAIR):
            for t in range(TOKC):
                ps = spsum.tile([P, 512], F32, tag="sc", name="psk")
                for c in range(DIN):
                    nc.tensor.matmul(
                        ps[:],
                        wk_t[:, c, pr * P:(pr + 1) * P],
                        xt_t[:, c, t * 512:(t + 1) * 512],
                        start=(c == 0), stop=(c == DIN - 1),
                    )
                nc.vector.tensor_scalar_add(
                    kt[pr][:, t * 512:(t + 1) * 512], ps[:], bk_t[:, pr:pr + 1]
                )
        wq_t = load_w(wq_r, "wq_t")

        # ---- V/Q projections + attention, per quad / pair ----
        for g in range(NQUAD):
            prs = (2 * g, 2 * g + 1)
            # V projection for the quad's 256 columns; evict 128 cols
            # into each pair tile (strided dest skips ones columns)
            for t in range(NTOK):
                ps = ppsum.tile([P, 256], F32, tag="proj", name="psv")
                for c in range(DIN):
                    nc.tensor.matmul(
                        ps[:],
                        xt_t[:, c, t * P:(t + 1) * P],
                        wv_t[:, c, g * 256:(g + 1) * 256],
                        start=(c == 0), stop=(c == DIN - 1),
                    )
                for u in range(2):
                    dst = vt[prs[u]].rearrange(
                        "p t (h c) -> p t h c", c=65)[:, t, :, 0:64]
                    src = ps[:, u * 128:(u + 1) * 128].rearrange(
                        "p (h c) -> p h c", c=64)
                    bsrc = bvb[:, (g * 256 + u * 128):
                               (g * 256 + (u + 1) * 128)
                               ].rearrange("p (h c) -> p h c", c=64)
                    nc.vector.tensor_add(dst, src, bsrc)
            # Q^T projection for the quad's two pairs (all q chunks)
            for pr in prs:
                for t in range(QCN):
                    ps = ppsum.tile([P, 512], F32, tag="proj",
                                    name="psq")
                    for c in range(DIN):
                        nc.tensor.matmul(
                            ps[:],
                            wq_t[:, c, pr * P:(pr + 1) * P],
                            xt_t[:, c, t * 512:(t + 1) * 512],
                            start=(c == 0), stop=(c == DIN - 1),
                        )
                    nc.vector.tensor_scalar_add(
                        qt[pr][:, t * 512:(t + 1) * 512], ps[:],
                        bq_t[:, pr:pr + 1]
                    )

            # ---- attention for each pair of the quad ----
            for pr in prs:
                vpr = vt[pr].rearrange("p t (h c) -> p t h c", c=65)
                for qc in range(QCN):
                    qsl = slice(qc * 512, (qc + 1) * 512)
                    za = zpsum.tile([P, 512], F32, tag="z", name=f"za{pr}_{qc}")
                    zb = zpsum.tile([P, 512], F32, tag="z", name=f"zb{pr}_{qc}")
                    for kc in range(KC):
                        sq = spsum.tile([P, 2, 512], F32, tag="sc", name="sq")
                        nc.tensor.matmul(
                            sq[:, 0, :],
                            kt[pr][0:64, kc * P:(kc + 1) * P],
                            qt[pr][0:64, qsl],
                            start=True, stop=True,
                        )
                        nc.tensor.matmul(
                            sq[:, 1, :],
                            kt[pr][64:128, kc * P:(kc + 1) * P],
                            qt[pr][64:128, qsl],
                            start=True, stop=True,
                        )
                        pq = probs_pool.tile([P, 2, 512], BF16, tag="probs",
                                             name="pq")
                        nc.scalar.activation(pq[:], sq[:], EXP, scale=0.125)
                        nc.tensor.matmul(
                            za[0:65, :], vpr[:, kc, 0, :], pq[:, 0, :],
                            start=(kc == 0), stop=(kc == KC - 1),
                        )
                        nc.tensor.matmul(
                            zb[0:65, :], vpr[:, kc, 1, :], pq[:, 1, :],
                            start=(kc == 0), stop=(kc == KC - 1),
                        )
                    # normalize + evict Z^T; 1/rowsum partition-broadcast
                    # goes through a DRAM bounce (SBUF->SBUF bcast illegal)
                    rcin = work.tile([P, 2, 512], F32, tag="rcin", name="rcin", bufs=2)
                    rsc = dramp.tile([2, 512], F32, tag="rsc", name="rsc")
                    # Stage rowsum rows to SBUF via ScalarE (the approx-recip
                    # custom-DVE op misreads PSUM on HW), DRAM-bounce them to
                    # a partition-broadcast tile, then take the reciprocal
                    # there (the custom op also requires base partition 0).
                    nc.scalar.copy(rcin[64:65, 0, :], za[64:65, :])
                    nc.vector.tensor_copy(rcin[64:65, 1, :], zb[64:65, :])
                    nc.sync.dma_start(rsc[0:1, :], rcin[64:65, 0, :])
                    nc.sync.dma_start(rsc[1:2, :], rcin[64:65, 1, :])
                    rbr = work.tile([P, 2, 512], F32, tag="rbc", name="rbr")
                    nc.sync.dma_start(rbr[0:64, 0, :],
                                      rsc[0:1, :].to_broadcast((64, 512)))
                    nc.sync.dma_start(rbr[0:64, 1, :],
                                      rsc[1:2, :].to_broadcast((64, 512)))
                    rb = work.tile([P, 2, 512], F32, tag="rbc", name="rb")
                    nc.vector.reciprocal_approx_fast(rb[0:64, :, :],
                                                     rbr[0:64, :, :])
                    nc.vector.tensor_mul(zt[pr][0:64, qsl], za[0:64, :],
                                         rb[0:64, 0, :])
                    zs = work.tile([P, 512], BF16, tag="zstage", name="zs")
                    nc.vector.tensor_mul(zs[0:64, :], zb[0:64, :],
                                         rb[0:64, 1, :])
                    nc.sync.dma_start(zt[pr][64:128, qsl], zs[0:64, :])

        # ---- output projection ----
        wo_t = load_w(wo_r, "wo_t")
        for qc in range(QCN):
            oqsl = slice(qc * 512, (qc + 1) * 512)
            for ec in range(DIN):
                ps = spsum.tile([P, 512], F32, tag="sc", name="pso")
                for dc in range(NPAIR):
                    nc.tensor.matmul(
                        ps[:],
                        wo_t[:, dc, ec * P:(ec + 1) * P],
                        zt[dc][:, oqsl],
                        start=(dc == 0), stop=(dc == NPAIR - 1),
                    )
                st = work.tile([P, 512], F32, tag="stage", name="st", bufs=2)
                nc.vector.tensor_scalar_add(st[:], ps[:], bo_t[:, ec:ec + 1])
                nc.sync.dma_start(ot_d[ec * P:(ec + 1) * P, oqsl], st[:])

    nc.compile()
    return nc



# ---------------- host-side entry point ----------------

BF = ml_dtypes.bfloat16
_B, _S, _D, _H = 4, 2048, 1024, 16
_SQ = _S // 2
_NC_CACHE = None


def _get_nc():
    global _NC_CACHE
    if _NC_CACHE is None:
        _NC_CACHE = build_attention_nc(S_full=_S, SQ=_SQ, D=_D, H=_H)
    return _NC_CACHE


def kernel(X, Wq, bq, Wk, bk, Wv, bv, Wo, bo):
    """Full-input multi-head attention on 8 TRN2 NeuronCores.

    Sharding: core c handles batch c//2, query-half c%2 (no collectives;
    K/V are recomputed per query-half). Inputs are cast to bf16 on host
    (matmul precision), X is transposed per core with its query half
    leading; output is fp32 [B, S, D].
    """
    from concourse.bass_utils import run_bass_kernel_spmd

    X = np.asarray(X, dtype=np.float32)
    bq = np.asarray(bq, dtype=np.float32)
    bk = np.asarray(bk, dtype=np.float32)
    bv = np.asarray(bv, dtype=np.float32)
    bo = np.asarray(bo, dtype=np.float32)
    wqb = np.ascontiguousarray(np.asarray(Wq, dtype=np.float32).astype(BF))
    wkb = np.ascontiguousarray(np.asarray(Wk, dtype=np.float32).astype(BF))
    wvb = np.ascontiguousarray(np.asarray(Wv, dtype=np.float32).astype(BF))
    wob = np.ascontiguousarray(np.asarray(Wo, dtype=np.float32).astype(BF))

    in_maps = []
    for c in range(8):
        b, half = c // 2, c % 2
        order = np.concatenate([
            np.arange(half * _SQ, (half + 1) * _SQ),
            np.arange((1 - half) * _SQ, (2 - half) * _SQ),
        ])
        xtp = np.ascontiguousarray(X[b][order, :].T.astype(BF))
        in_maps.append({
            "xt": xtp, "wq": wqb, "wk": wkb, "wv": wvb, "wo": wob,
            "bq": bq, "bk": bk, "bv": bv, "bo": bo,
        })

    nc = _get_nc()
    res = run_bass_kernel_spmd(nc, in_maps, list(range(8)))

    out = np.empty((_B, _S, _D), np.float32)
    for c in range(8):
        b, half = c // 2, c % 2
        out[b, half * _SQ:(half + 1) * _SQ, :] = res.results[c]["ot"].T
    return out

